# revision 14
# baseline (speedup 1.0000x reference)
# Trainium2 Bass kernel for nn_Attention (4x2048x1024, H=16, DH=64) on 8 NeuronCores.
#
# Sharding: core c = 2*bi + g handles batch bi (2048 tokens) and head group g
# (8 of 16 heads). Per-core: x @ Wqkv slice -> per-head attention -> partial
# MLP with W_mlp rows for its heads; host sums the two partials per batch and
# adds the bias.
#
# Layout/schedule notes:
# - x arrives HOST-PRE-TRANSPOSED as xT [DIM, TOK] bf16, so no PE transposes
#   or staging: one DMA straight into the persistent xT tile.
# - Q^T/K^T [dh, tok] per head pair (partitions 0:64 = even head, 64:128 =
#   odd head) via W-stationary matmuls over 8 k-tiles.
# - S^T = K.T @ Q per (head pair, key tile) with row-paired matmuls via
#   tile_position; exp on ACT (scale=1/8, no max subtraction; act table
#   pre-warmed) writes FP8 directly into a per-kt-pair tile laid out
#   [p, (head, ktparity, q)].
# - PV runs in fp8 DoubleRow: stationary V for a kt PAIR [128, 2, 65] (65th
#   col = ones -> softmax denominator in psum row 64), contraction K=256 per
#   matmul at 0.5 cyc/col. V is SPLIT V = V_hi + V_lo (both fp8, both
#   accumulated) so V keeps ~fp16 precision; only P's fp8 error remains
#   (rel err ~1.4e-2 < 2e-2 gate, measured numerically).
# - Softmax denominator: PE broadcast matmul of psum row 64 against a ones
#   column, reciprocal_approx_fast, then one DVE multiply.
# - MLP pairs heads (K=128): even head's normalized attn written to
#   partitions 0:64 of a pair tile, odd head's moved to partitions 64:128 via
#   SBUF->SBUF DMA (DVE cannot shift partitions).
# - Schedule: the ACT engine (256 exps of [128,1024], ~1.1us each) is the
#   wall; S+exp+PV are emitted under tc.high_priority() so the PE always
#   prefers feeding ACT, with K/Q/V fills and MLP chunks as slack-fillers.
#   Each unit's den/normalize is deferred into the next unit's kt loop so
#   ACT never idles at unit boundaries; Q chunks for qc2/3 fill the PE-light
#   later units. psum: S 2x[128,1024] + poA/poB + pb + pm = 8 banks.
# - reps>1 runs the body under For_i_unrolled(max_unroll=4): the all-engine
#   barrier lands every 4 reps so the next body's DMA/QKV head overlaps the
#   previous body's attention tail.
import ml_dtypes
import numpy as np
import concourse.bass as bass
import concourse.mybir as mybir
import concourse.tile as tile
from concourse import bacc, bass_utils

f32 = mybir.dt.float32
f32r = mybir.dt.float32r
bf16 = mybir.dt.bfloat16
fp8 = mybir.dt.float8e4
AF = mybir.ActivationFunctionType
DR = mybir.MatmulPerfMode.DoubleRow

TOK = 2048
DIM = 1024
NH = 8          # heads per core
DH = 64
FEAT = NH * DH  # 512
KT = DIM // 128     # 8 k-tiles over dim
TT = TOK // 128     # 16 token tiles
DKT = TT // 2       # 8 kt-pairs
NQC = TOK // 512    # 4 q/tok chunks
HP = NH // 2        # 4 head pairs


def build(reps=1):
    nc = bacc.Bacc("TRN2", target_bir_lowering=False, debug=False)
    xTd = nc.dram_tensor("xT", [DIM, TOK], bf16, kind="ExternalInput").ap()
    wq = nc.dram_tensor("wq", [DIM, FEAT], bf16, kind="ExternalInput").ap()
    wk = nc.dram_tensor("wk", [DIM, FEAT], bf16, kind="ExternalInput").ap()
    wv = nc.dram_tensor("wv", [DIM, FEAT], bf16, kind="ExternalInput").ap()
    wm = nc.dram_tensor("wm", [FEAT, DIM], bf16, kind="ExternalInput").ap()
    outT = nc.dram_tensor("outT", [DIM, TOK], bf16, kind="ExternalOutput").ap()

    with tile.TileContext(nc) as tc:
        with tc.tile_pool(name="const", bufs=1) as constp, \
             tc.tile_pool(name="pers", bufs=1) as pers, \
             tc.tile_pool(name="work", bufs=1) as work, \
             tc.tile_pool(name="ps", bufs=1, space="PSUM") as psp:
            # ---- constants ----
            ones_f = constp.tile([128, 64], f32)
            nc.gpsimd.memset(ones_f[:], 1.0)
            onesr = constp.tile([128, 64], f32r)
            nc.vector.tensor_copy(onesr[:], ones_f[:])
            ones8 = constp.tile([128, 16], fp8)
            nc.vector.tensor_copy(ones8[:], ones_f[:, 0:16])
            zero8 = constp.tile([128, 16], fp8)
            nc.gpsimd.memset(zero8[:], 0.0)

            # ---- persistent tiles ----
            # xT consolidated: [p, (k t)]; slice k gives the [128, TOK] k-tile.
            xT = pers.tile([128, KT * TOK], bf16, tag="xT", name="xT")
            xTv = xT[:].rearrange("p (k t) -> p k t", t=TOK)
            wq_all = pers.tile([128, KT * FEAT], bf16, tag="wq", name="wq_all")
            wk_all = pers.tile([128, KT * FEAT], bf16, tag="wk", name="wk_all")
            wv_all = pers.tile([128, KT * FEAT], bf16, tag="wv", name="wv_all")
            wm_all = pers.tile([128, HP * DIM], bf16, tag="wm", name="wm_all")
            QT = [pers.tile([128, TOK], bf16, tag=f"QT{i}", name=f"QT{i}")
                  for i in range(HP)]
            KTt = [pers.tile([128, TOK], bf16, tag=f"KT{i}", name=f"KT{i}")
                   for i in range(HP)]
            # V for DoubleRow: per kt-pair [p, (j, h, 128)], hi + lo fp8
            # splits. Padded to 128 cols per head: the ISA requires DR
            # ldweights to target all 4 col groups (col_grp==0xf) and a
            # 16-aligned pair stride; cols 65:128 are don't-care (psum rows
            # 65:128 are never read).
            VH = [pers.tile([128, 2 * NH * 128], fp8, tag=f"VH{i}", name=f"VH{i}")
                  for i in range(DKT)]
            VL = [pers.tile([128, 2 * NH * 128], fp8, tag=f"VL{i}", name=f"VL{i}")
                  for i in range(DKT)]
            # zero via DVE (keeps the Pool/SWDGE queue free for weight DMAs),
            # then ones into hi col 64 -- the den column. Init-time only; the
            # rep body rewrites just cols 0:64.
            zbig = constp.tile([128, 2 * NH * 128], fp8)
            nc.gpsimd.memset(zbig[:], 0.0)
            for dk in range(DKT):
                nc.vector.tensor_copy(VH[dk][:], zbig[:])
                nc.vector.tensor_copy(VL[dk][:], zbig[:])
                vh = VH[dk][:].rearrange("p (g e) -> p g e", e=128)
                nc.vector.tensor_copy(
                    vh[:, :, 64:65],
                    ones8[:].rearrange("p (g e) -> p g e", e=1))
            # preload the Exp act table so the first real exp skips the
            # ~1.3us table load
            warm = constp.tile([128, 2], f32)
            nc.scalar.activation(warm[:], ones_f[:, 0:2], AF.Exp)

            def emit_body():
                # ==== input DMA: xT on the sync queue; weight DMAs on the
                #      gpsimd SWDGE queue so the ACT sequencer stays free ====
                nc.sync.dma_start(
                    out=xTv,
                    in_=xTd[:].rearrange("(k p) t -> p k t", p=128))
                for src, dst in ((wk, wk_all), (wv, wv_all), (wq, wq_all)):
                    nc.gpsimd.dma_start(
                        out=dst[:].rearrange("p (k c) -> p k c", c=FEAT),
                        in_=src[:].rearrange("(k p) c -> p k c", p=128))
                nc.gpsimd.dma_start(
                    out=wm_all[:].rearrange("p (h d) -> p h d", d=DIM),
                    in_=wm[:].rearrange("(h p) d -> p h d", p=128))

                # Phase-1 psum rotates over pb+pm only: attention needs
                # s/poA/poB immediately, while pb's first use is one full unit
                # in and pm's is a full q-chunk in.
                p1tags = ["pb", "pm"]
                p1n = [0]

                def p1tile(shape, dtype):
                    t = psp.tile(shape, dtype, tag=p1tags[p1n[0] % 2],
                                 name="p1")
                    p1n[0] += 1
                    return t

                # ==== K^T, Q^T, V for one head pair (emitted just-in-time
                #      between attention units so exp starts early) ====
                def emit_kq_chunk(W_all, dstl, f, qc):
                    pq = p1tile([128, 512], f32)
                    for k in range(KT):
                        nc.tensor.matmul(
                            pq[:],
                            W_all[:, k * FEAT + f * 128:k * FEAT + (f + 1) * 128],
                            xTv[:, k, qc * 512:(qc + 1) * 512],
                            start=(k == 0), stop=(k == KT - 1))
                    nc.vector.tensor_copy(
                        dstl[f][:, qc * 512:(qc + 1) * 512], pq[:])

                def emit_v(f, tt):
                    pv = p1tile([128, 128], f32)
                    for k in range(KT):
                        nc.tensor.matmul(
                            pv[:], xTv[:, k, tt * 128:(tt + 1) * 128],
                            wv_all[:, k * FEAT + f * 128:k * FEAT + (f + 1) * 128],
                            start=(k == 0), stop=(k == KT - 1))
                    dh = VH[tt // 2][:].rearrange("p (j h e) -> p j h e", j=2, e=128)
                    dl = VL[tt // 2][:].rearrange("p (j h e) -> p j h e", j=2, e=128)
                    hi = dh[:, tt % 2, 2 * f:2 * f + 2, 0:64]
                    nc.vector.tensor_copy(
                        hi, pv[:].rearrange("p (h e) -> p h e", e=64))
                    nc.vector.tensor_sub(
                        dl[:, tt % 2, 2 * f:2 * f + 2, 0:64],
                        pv[:].rearrange("p (h e) -> p h e", e=64), hi)

                def emit_kqv_min(f):
                    # prefix for unit (0, f): all of K, Q chunks 0-1; V is
                    # emitted just-in-time inside the unit's kt loop
                    for qc in range(NQC):
                        emit_kq_chunk(wk_all, KTt, f, qc)
                    emit_kq_chunk(wq_all, QT, f, 0)
                    emit_kq_chunk(wq_all, QT, f, 1)
                    emit_v(f, 0)
                    emit_v(f, 1)

                # Later head pairs' K/Q/V are queued as fill work and drained
                # two chunks per key tile inside the units that precede their
                # first use, so the PE never sees a projection burst between
                # units (which would starve the exp engine).
                pending_fill = []

                def push_kqv_min(f):
                    for qc in range(NQC):
                        pending_fill.append(
                            lambda f=f, qc=qc: emit_kq_chunk(wk_all, KTt, f, qc))
                    pending_fill.append(lambda f=f: emit_kq_chunk(wq_all, QT, f, 0))
                    pending_fill.append(lambda f=f: emit_kq_chunk(wq_all, QT, f, 1))
                    pending_fill.append(lambda f=f: emit_v(f, 0))
                    pending_fill.append(lambda f=f: emit_v(f, 1))

                # Q chunks for qc2/qc3 are deferred into the PE-light later
                # units (one unit ahead of their consumer): phase 1 (units
                # 1-8) is PE-oversubscribed (K/V/Q fills + attention exceeds
                # the ACT pace there), phase 2 has slack.
                pre_unit = {}
                pre_unit[(0, 0)] = lambda: (emit_kqv_min(0), push_kqv_min(1))
                pre_unit[(1, 0)] = lambda: push_kqv_min(2)
                pre_unit[(0, 1)] = lambda: push_kqv_min(3)
                pre_unit[(1, 3)] = lambda: emit_kq_chunk(wq_all, QT, 0, 2)
                pre_unit[(2, 0)] = lambda: emit_kq_chunk(wq_all, QT, 1, 2)
                pre_unit[(2, 1)] = lambda: emit_kq_chunk(wq_all, QT, 2, 2)
                pre_unit[(2, 2)] = lambda: emit_kq_chunk(wq_all, QT, 3, 2)
                pre_unit[(2, 3)] = lambda: emit_kq_chunk(wq_all, QT, 0, 3)
                pre_unit[(3, 0)] = lambda: emit_kq_chunk(wq_all, QT, 1, 3)
                pre_unit[(3, 1)] = lambda: emit_kq_chunk(wq_all, QT, 2, 3)
                pre_unit[(3, 2)] = lambda: emit_kq_chunk(wq_all, QT, 3, 3)

                # qc0/qc1 interleaved so each KQV block feeds two units of
                # exp work; later q-chunks run pure attention.
                unit_order = [(0, 0), (1, 0), (0, 1), (1, 1), (0, 2), (1, 2),
                              (0, 3), (1, 3), (2, 0), (2, 1), (2, 2), (2, 3),
                              (3, 0), (3, 1), (3, 2), (3, 3)]

                # ==== attention + MLP (MLP chunks deferred one q-chunk and
                #      interleaved into the next chunk's units) ====
                pending_mlp = []

                def emit_mlp(qc, m, arp_l, ptag="pm"):
                    pm = psp.tile([128, 512], f32, tag=ptag, name="pm")
                    for hp in range(HP):
                        nc.tensor.matmul(
                            pm[:],
                            wm_all[:, hp * DIM + m * 128:hp * DIM + (m + 1) * 128],
                            arp_l[hp][:],
                            start=(hp == 0), stop=(hp == HP - 1))
                    ev = work.tile([128, 512], bf16, tag="ev", name="ev", bufs=4)
                    nc.vector.tensor_copy(ev[:], pm[:])
                    nc.sync.dma_start(
                        out=outT[m * 128:(m + 1) * 128, qc * 512:(qc + 1) * 512],
                        in_=ev[:])

                arps = {qc: [None] * HP for qc in range(NQC)}
                ndone = {qc: 0 for qc in range(NQC)}

                # den/normalize of unit i is deferred into unit i+1's kt
                # loop (popped right after the first exp) so the next unit's
                # S/exp stream starts before the boundary bookkeeping and
                # ACT never idles between units.
                pending_norm = []

                def emit_norm(qc, hp, poA, poB):
                    arp = arps[qc]
                    hA, hB = 2 * hp, 2 * hp + 1
                    ar = work.tile([128, 512], bf16, tag=f"arp{hp}",
                                   name=f"arp{hp}", bufs=3)
                    arp[hp] = ar
                    for h, po in ((hA, poA), (hB, poB)):
                        tmp = work.tile([65, 512], f32r, tag="tmp",
                                        name="tmp", bufs=4)
                        nc.vector.tensor_copy(tmp[:], po[0:65, :])
                        pb = psp.tile([64, 512], f32, tag="pb", name="pb")
                        nc.tensor.matmul(pb[:], onesr[64:65, 0:64],
                                         tmp[64:65, :], start=True,
                                         stop=True)
                        rc = work.tile([64, 512], f32, tag="rc", name="rc",
                                       bufs=2)
                        nc.vector.reciprocal_approx_fast(out=rc[:],
                                                         in_=pb[:])
                        if h == hA:
                            nc.vector.tensor_mul(ar[0:64, :], tmp[0:64, :],
                                                 rc[:])
                        else:
                            arB = work.tile([64, 512], bf16, tag="arB",
                                            name="arB", bufs=2)
                            nc.vector.tensor_mul(arB[:], tmp[0:64, :],
                                                 rc[:])
                            nc.gpsimd.dma_start(out=ar[64:128, :], in_=arB[:])
                    ndone[qc] += 1
                    if ndone[qc] == HP:
                        for m in range(KT):
                            pending_mlp.append(
                                (lambda qc=qc, m=m, arp_l=list(arp), **kw:
                                 emit_mlp(qc, m, arp_l, **kw)))

                for qc, hp in unit_order:
                    if (qc, hp) in pre_unit:
                        pre_unit[(qc, hp)]()
                    hA, hB = 2 * hp, 2 * hp + 1
                    poA = psp.tile([128, 512], f32, tag="poA", name="poA")
                    poB = psp.tile([128, 512], f32, tag="poB", name="poB")
                    pt8 = None
                    for kt in range(TT):
                        if qc == 0 and kt < TT - 2:
                            emit_v(hp, kt + 2)
                        if kt % 2 == 1:
                            if pending_fill:
                                pending_fill.pop(0)()
                            elif pending_mlp:
                                pending_mlp.pop(0)()
                        dkt, j = kt // 2, kt % 2
                        if j == 0:
                            pt8 = work.tile([128, 2048], fp8, tag="pt8",
                                            name="pt8", bufs=3)
                        ptv = pt8[:].rearrange("p (h j q) -> p h j q", h=2, j=2)
                        ps_s = psp.tile([128, 1024], f32, tag="s", name="ps_s",
                                        bufs=2)
                        # S+exp+PV get scheduler priority over fills/MLP so
                        # the PE always prefers feeding the ACT engine (the
                        # wall); fills run in whatever slack remains.
                        with tc.high_priority():
                            nc.tensor.matmul(
                                ps_s[:, 0:512],
                                KTt[hp][0:64, kt * 128:(kt + 1) * 128],
                                QT[hp][0:64, qc * 512:(qc + 1) * 512],
                                start=True, stop=True, tile_position=(0, 0))
                            nc.tensor.matmul(
                                ps_s[:, 512:1024],
                                KTt[hp][64:128, kt * 128:(kt + 1) * 128],
                                QT[hp][64:128, qc * 512:(qc + 1) * 512],
                                start=True, stop=True, tile_position=(64, 0))
                            nc.scalar.activation(ptv[:, :, j, :], ps_s[:],
                                                 AF.Exp, scale=0.125)
                        if kt == 0 and pending_norm:
                            pending_norm.pop(0)()
                        if j == 1:
                            vh = VH[dkt][:].rearrange("p (j h e) -> p j h e",
                                                      j=2, e=128)
                            vl = VL[dkt][:].rearrange("p (j h e) -> p j h e",
                                                      j=2, e=128)
                            with tc.high_priority():
                                for h, po in ((hA, poA), (hB, poB)):
                                    nc.tensor.matmul(
                                        po[:], vh[:, :, h, :],
                                        ptv[:, h % 2, :, :],
                                        start=(dkt == 0), stop=False,
                                        perf_mode=DR)
                                    nc.tensor.matmul(
                                        po[:], vl[:, :, h, :],
                                        ptv[:, h % 2, :, :],
                                        start=False, stop=(dkt == DKT - 1),
                                        perf_mode=DR)
                        if kt % 4 == 3 and pending_mlp:
                            pending_mlp.pop(0)()
                    pending_norm.append(
                        lambda qc=qc, hp=hp, poA=poA, poB=poB:
                        emit_norm(qc, hp, poA, poB))
                while pending_norm:
                    pending_norm.pop(0)()
                # tail drain: alternate with the now-free poA/poB banks so
                # the last MLP chunks ping-pong instead of serializing
                dtags = ["pm", "poA", "poB", "pb"]
                di = 0
                while pending_mlp:
                    pending_mlp.pop(0)(ptag=dtags[di % 4])
                    di += 1

            if reps == 1:
                emit_body()
            else:
                # unrolled loop: all-engine barrier only every UNROLL reps
                tc.For_i_unrolled(0, reps, 1,
                                  lambda iv: emit_body(),
                                  max_unroll=4)
    nc.compile()
    return nc


_nc_cache = {}


def get_nc(reps=1):
    if reps not in _nc_cache:
        _nc_cache[reps] = build(reps)
    return _nc_cache[reps]


def make_in_maps(input, W_qkv, W_mlp):
    bf = ml_dtypes.bfloat16
    W_qkv = W_qkv.astype(bf)
    W_mlp = W_mlp.astype(bf)
    in_maps = []
    for c in range(8):
        bi, g = c // 2, c % 2
        cols = slice(g * FEAT, (g + 1) * FEAT)
        in_maps.append({
            "xT": np.ascontiguousarray(input[bi].T.astype(bf)),
            "wq": np.ascontiguousarray(W_qkv[:, 0 * DIM:1 * DIM][:, cols]),
            "wk": np.ascontiguousarray(W_qkv[:, 1 * DIM:2 * DIM][:, cols]),
            "wv": np.ascontiguousarray(W_qkv[:, 2 * DIM:3 * DIM][:, cols]),
            "wm": np.ascontiguousarray(W_mlp[g * FEAT:(g + 1) * FEAT, :]),
        })
    return in_maps


def kernel(input, W_qkv, W_mlp, b_mlp, reps=1):
    nc = get_nc(reps)
    in_maps = make_in_maps(np.asarray(input), np.asarray(W_qkv), np.asarray(W_mlp))
    res = bass_utils.run_bass_kernel_spmd(nc, in_maps, core_ids=list(range(8)))
    out = np.empty((4, TOK, DIM), np.float32)
    b = np.asarray(b_mlp)
    for bi in range(4):
        out[bi] = (res.results[2 * bi]["outT"].astype(np.float32)
                   + res.results[2 * bi + 1]["outT"].astype(np.float32)).T + b
    return out
